# revision 1
# baseline (speedup 1.0000x reference)
"""Multi-head attention (B=4, S=2048, D=1024, H=16, dk=dv=64) on 8 TRN2 cores.

Sharding: core c = 2*b + hg handles batch b = c//2 and heads
[hg*8, hg*8+8). Each core computes a partial output
(its 8 heads' contribution through Wo); the host adds the two partials
per batch.

Per-core device pipeline (matmul inputs bf16, PSUM accumulation fp32).
The kernel is issue-ordered so the list scheduler keeps the PE gap-free
(p-state at max) and the ACT exp stream (the second-busiest engine)
starts ~14us in and never starves:

  - k-proj block 0 and q-proj(qb0,p0) are issued first; attention
    (qb0,p0) scores begin immediately after.
  - all remaining projection work (k blocks 1-3, q pairs, v chunks) is
    issued as PE filler interleaved into the attention g-loops of qb0,
    so the PE always has ready work while exp(g) -> mix(g) dependencies
    drain. v chunk t is projected just-in-time before mix needs it.
  - for qb>0, the fillers are the previous qb's Wo matmuls and the next
    qb's q projection.
  - scores^T per head pair are K=64 matmuls on partition halves
    (h0: partitions 0:64 -> PE tile (0,0); h1: 64:128 -> tile (64,0)),
    [128 keys, 512 q] fp32 in PSUM, two key chunks per [128,1024] PSUM
    tile so each ScalarE exp instruction covers 2 banks.
  - mix^T + softmax sums in one matmul: lhsT = vh_aug [128 keys, 65]
    (col 64 = mask), rhs = exp chunk half; h0/h1 accumulate into the
    two banks of one [128,1024] PSUM tile over the 16 key chunks.
  - mix for group g is issued one g-iteration behind its exp so the PE
    never sits adjacent to the ACT dependency (weights prefetch).
  - normalize (PE-free): evacuate mix rows 0:65 to SBUF (frees the
    PSUM tile after one DVE op), bounce the sums row (partition 64)
    through a DRAM scratch tile to broadcast it over 64 partitions,
    reciprocal on DVE, scale multiplies on the idle GpSimd. The final
    pair instead uses a K=1 f32r PE-broadcast matmul + DVE multiplies
    straight out of PSUM (lowest latency - it is the kernel tail).
    h1's normalized tile is DMA-shifted to partitions 64-127 so each
    pair's mix^T is one [128, 512] tile (e on partitions).
  - out += mixT_norm.T @ Wo: dense K=128 bf16 matmuls accumulating
    over the 4 pairs; DVE evac fp32 -> DMA to HBM. The last qb's Wo
    runs p-major across 6 concurrent PSUM accumulators so only the
    final pair's 6 matmuls + evacs trail the last normalize.

PSUM: sc ring 2x[128,1024] (scores + qb0 projection accumulators +
the tail broadcast) = 4 banks, mix 1x[128,1024] = 2 banks, aux ring
2x[128,512] (projection pj + Wo accumulators) = 2 banks.
"""

import numpy as np

B, S, D = 4, 2048, 1024
H, DK, DV = 16, 64, 64
HC = 8          # heads per core
NP = HC // 2    # head pairs per core
NCORES = 8
NC_CHUNKS = D // 128    # 8 contraction chunks over D
NKC = S // 128          # 16 key chunks
NQB = S // 512          # 4 query blocks
VW = HC * (DV + 1)      # vh storage: 65 cols per head (dv | mask)

_COMPILED = {}


def _build_nc():
    import concourse.tile as tile
    from concourse import bacc, mybir
    from contextlib import ExitStack

    F32 = mybir.dt.float32
    F32R = mybir.dt.float32r
    BF16 = mybir.dt.bfloat16
    EXP = mybir.ActivationFunctionType.Exp
    COPY = mybir.ActivationFunctionType.Copy

    nc = bacc.Bacc("TRN2", target_bir_lowering=False, debug=False,
                   num_devices=NCORES)

    qT = nc.dram_tensor("qT", [D, S], BF16, kind="ExternalInput").ap()
    kT = nc.dram_tensor("kT", [D, S], BF16, kind="ExternalInput").ap()
    vT = nc.dram_tensor("vT", [D, S], BF16, kind="ExternalInput").ap()
    wq = nc.dram_tensor("wq", [D, HC * DK], BF16, kind="ExternalInput").ap()
    wk = nc.dram_tensor("wk", [D, HC * DK], BF16, kind="ExternalInput").ap()
    wv = nc.dram_tensor("wv", [D, HC * DV], BF16, kind="ExternalInput").ap()
    wo = nc.dram_tensor("wo", [HC * DV, D], BF16, kind="ExternalInput").ap()
    maskr = nc.dram_tensor("maskr", [128, NKC], F32, kind="ExternalInput").ap()
    out = nc.dram_tensor("out", [S, D], F32, kind="ExternalOutput").ap()

    with tile.TileContext(nc) as tc:
        with ExitStack() as ctx:
            const_pool = ctx.enter_context(tc.tile_pool(name="const", bufs=1))
            w_pool = ctx.enter_context(tc.tile_pool(name="weights", bufs=1))
            act_pool = ctx.enter_context(tc.tile_pool(name="acts", bufs=1))
            # 5 blocks of 8 stage tiles live at once (k0-k3 + q0); later
            # q blocks wrap onto long-drained slots
            st_pool = ctx.enter_context(
                tc.tile_pool(name="stage", bufs=5 * NC_CHUNKS))
            vt_pool = ctx.enter_context(tc.tile_pool(name="vstage", bufs=2))
            exp_pool = ctx.enter_context(tc.tile_pool(name="exp", bufs=6))
            norm_pool = ctx.enter_context(tc.tile_pool(name="norm", bufs=2 * NP))
            rec_pool = ctx.enter_context(tc.tile_pool(name="rec", bufs=4))
            sums_pool = ctx.enter_context(tc.tile_pool(name="sums", bufs=2))
            sh_pool = ctx.enter_context(tc.tile_pool(name="sh", bufs=4))
            osb_pool = ctx.enter_context(tc.tile_pool(name="outsb", bufs=4))
            sc_pool = ctx.enter_context(
                tc.tile_pool(name="scpsum", bufs=2, space="PSUM"))
            mix_pool = ctx.enter_context(
                tc.tile_pool(name="mxpsum", bufs=1, space="PSUM"))
            aux_pool = ctx.enter_context(
                tc.tile_pool(name="auxpsum", bufs=2, space="PSUM"))
            dram_pool = ctx.enter_context(
                tc.tile_pool(name="dscratch", bufs=4, space="DRAM"))

            mask_sb = const_pool.tile([128, NKC], F32)
            nc.sync.dma_start(mask_sb[:], maskr[:])
            # PE warmup: the PE would otherwise idle ~13us waiting for
            # the first weight/staging DMAs, then pay the 0.65/1.2GHz
            # p-state ramp on the real projections. Dummy matmuls on
            # resident SBUF data bridge the wait at full ramp.
            warm_sb = const_pool.tile([128, 512], BF16)
            nc.vector.memset(warm_sb[:], 0.0)
            warm_ps = aux_pool.tile([128, 512], F32, tag="aux",
                                    name="warmps")
            for i in range(48):
                nc.tensor.matmul(warm_ps[:], lhsT=warm_sb[:, 0:128],
                                 rhs=warm_sb[:],
                                 start=(i == 0), stop=(i == 47))
            warm_out = const_pool.tile([128, 512], F32)
            nc.vector.tensor_copy(warm_out[:], warm_ps[:])
            ones_sb = const_pool.tile([128, HC], BF16)
            nc.vector.memset(ones_sb[:], 1.0)
            # K=1 f32r ones row for the tail's PE sum-broadcast
            # (memset cannot write f32r; bounce through an f32 tile)
            e_ones32 = const_pool.tile([65, 64], F32)
            nc.vector.memset(e_ones32[:], 1.0)
            e_ones = const_pool.tile([65, 64], F32R)
            nc.vector.tensor_copy(e_ones[:], e_ones32[:])

            # DMA priority order: the head's critical path is
            # wk + k staging (first projections), then wq + q staging;
            # wv is needed at attention start and wo only at qb1.
            wq_sb = w_pool.tile([128, NC_CHUNKS * 512], BF16, tag="wq")
            wk_sb = w_pool.tile([128, NC_CHUNKS * 512], BF16, tag="wk")
            wv_sb = w_pool.tile([128, NC_CHUNKS * 512], BF16, tag="wv")
            wo_sb = w_pool.tile([128, NP * 1024], BF16, tag="wo")
            # head critical path: wk then kT-block0, whole chunks (1KB
            # partition lines - smaller splits lose DMA efficiency)
            for c in range(NC_CHUNKS):
                nc.sync.dma_start(wk_sb[:, c * 512:(c + 1) * 512],
                                  wk[c * 128:(c + 1) * 128, :])

            # persistent activations. khT/vh are single tiles (written
            # in slices, region-tracked) so consecutive attention
            # matmuls switch lhsT within one tile - cheaper LDWEIGHTS.
            qhTb = [[act_pool.tile([128, 512], BF16, tag=f"qhT{p}_{b}",
                                   name=f"qhT{p}_{b}") for b in range(NQB)]
                    for p in range(NP)]
            khT = [act_pool.tile([128, S], BF16, tag=f"khT{p}",
                                 name=f"khT{p}") for p in range(NP)]
            khTb = [[khT[p][:, b * 512:(b + 1) * 512] for b in range(NQB)]
                    for p in range(NP)]
            vhs_all = act_pool.tile([128, NKC * VW], BF16, tag="vhall")
            vhs = [vhs_all[:, t * VW:(t + 1) * VW] for t in range(NKC)]

            def stage_block(src, blk, split=False):
                stg = []
                for c in range(NC_CHUNKS):
                    t = st_pool.tile([128, 512], BF16, tag="stage",
                                     name=f"stg{c}")
                    nsp = (4 if c < 2 else 2) if split else 1
                    w = 512 // nsp
                    for h in range(nsp):
                        nc.sync.dma_start(
                            t[:, h * w:(h + 1) * w],
                            src[c * 128:(c + 1) * 128,
                                blk * 512 + h * w:blk * 512 + (h + 1) * w])
                    stg.append(t)
                return stg

            def proj_pair(stg, wsb, dst_tile, p, pool=None, act_evac=False):
                if pool is None:
                    ps = aux_pool.tile([128, 512], F32, tag="aux")
                else:
                    # borrow half an sc-ring tile (qb0 fillers: the sc
                    # ring has slack while attention is PE-bound)
                    pst = pool.tile([128, 1024], F32, tag="sc", name="pjsc")
                    ps = pst[:, 0:512]
                for c in range(NC_CHUNKS):
                    nc.tensor.matmul(
                        ps[:],
                        lhsT=wsb[:, c * 512 + p * 128:
                                 c * 512 + (p + 1) * 128],
                        rhs=stg[c][:],
                        start=(c == 0), stop=(c == NC_CHUNKS - 1))
                if act_evac:
                    # qb0 era: ScalarE is idle; keep DVE off the critical
                    # path of PSUM slot recycling
                    nc.scalar.activation(dst_tile[:], ps[:], COPY)
                else:
                    nc.vector.tensor_copy(dst_tile[:], ps[:])

            # vT is staged in 512-token quarters (large 1KB-line DMAs,
            # double buffered) so v-proj matmuls never wait on staging
            vt_cur = {}

            def stage_vq(vq):
                vt = vt_pool.tile([128, NC_CHUNKS * 512], BF16, tag="vt",
                                  name=f"vq{vq}")
                for c in range(NC_CHUNKS):
                    nc.sync.dma_start(
                        vt[:, c * 512:(c + 1) * 512],
                        vT[c * 128:(c + 1) * 128,
                           vq * 512:(vq + 1) * 512])
                vt_cur[vq] = vt

            def vproj_chunk(t):
                vt = vt_cur[t // 4]
                o = t % 4
                ps = aux_pool.tile([128, 512], F32, tag="aux")
                for c in range(NC_CHUNKS):
                    nc.tensor.matmul(
                        ps[:],
                        lhsT=vt[:, c * 512 + o * 128:
                                c * 512 + (o + 1) * 128],
                        rhs=wv_sb[:, c * 512:(c + 1) * 512],
                        start=(c == 0), stop=(c == NC_CHUNKS - 1))
                dst_dv = vhs[t][:, 0:VW].rearrange(
                    "p (h x) -> p h x", x=DV + 1)[:, :, 0:DV]
                src_dv = ps[:].rearrange("p (h x) -> p h x", x=DV)
                nc.scalar.activation(dst_dv, src_dv, COPY,
                                     scale=mask_sb[:, t:t + 1])
                dst_m = vhs[t][:, 0:VW].rearrange(
                    "p (h x) -> p h x", x=DV + 1)[:, :, DV:DV + 1]
                src_m = ones_sb[:, 0:HC].rearrange("p (h x) -> p h x", x=1)
                nc.vector.tensor_scalar_mul(dst_m, src_m,
                                            mask_sb[:, t:t + 1])

            # Wo for one (qb, tt, dh) triple, issued one matmul at a time
            # (4 accumulating steps + evac) so it spreads as PE filler
            wo_state = {}

            def wo_single(qb, normT, tt, dh, p):
                if p == 0:
                    wo_state[(tt, dh)] = aux_pool.tile(
                        [128, 512], F32, tag="aux", name=f"wps{tt}{dh}")
                wps = wo_state[(tt, dh)]
                nc.tensor.matmul(
                    wps[:],
                    lhsT=normT[p][:, tt * 128:(tt + 1) * 128],
                    rhs=wo_sb[:, p * 1024 + dh * 512:
                              p * 1024 + (dh + 1) * 512],
                    start=(p == 0), stop=(p == NP - 1))
                if p == NP - 1:
                    osb = osb_pool.tile([128, 512], F32, tag="osb")
                    nc.vector.tensor_copy(osb[:], wps[:])
                    nc.sync.dma_start(
                        out[qb * 512 + tt * 128:qb * 512 + (tt + 1) * 128,
                            dh * 512:(dh + 1) * 512], osb[:])

            def wo_piece(qb, normT, tt, dh):
                for p in range(NP):
                    wo_single(qb, normT, tt, dh, p)

            # ---- stage + project k block 0 / q block 0 for pair 0 ----
            kstg = [None] * NQB
            qstg = [None] * NQB
            kstg[0] = stage_block(kT, 0)
            for c in range(NC_CHUNKS):
                nc.sync.dma_start(wq_sb[:, c * 512:(c + 1) * 512],
                                  wq[c * 128:(c + 1) * 128, :])
            qstg[0] = stage_block(qT, 0)
            for c in range(NC_CHUNKS):
                nc.sync.dma_start(wv_sb[:, c * 512:(c + 1) * 512],
                                  wv[c * 128:(c + 1) * 128, :])
            proj_pair(kstg[0], wk_sb, khTb[0][0], 0, act_evac=True)
            proj_pair(qstg[0], wq_sb, qhTb[0][0], 0, act_evac=True)
            stage_vq(0)
            stage_vq(1)
            for p in range(NP):
                nc.sync.dma_start(wo_sb[:, p * 1024:(p + 1) * 1024],
                                  wo[p * 128:(p + 1) * 128, :])

            # filler thunks, one consumed at the top of each attention
            # g-iteration. The order guarantees every tile's write is
            # issued before its first read (pair p's blocks land during
            # pair p-1's loop, with p0's later k blocks interleaved
            # just ahead of the scores that need them).
            def kf(kb, p):
                return lambda: proj_pair(kstg[kb], wk_sb, khTb[p][kb], p,
                                         pool=sc_pool, act_evac=True)

            def qf(qb, p, pool=None):
                return lambda: proj_pair(qstg[qb], wq_sb, qhTb[p][qb], p,
                                         pool=pool,
                                         act_evac=pool is not None)

            # q-projection issued one matmul at a time (steady-state
            # filler granularity; the aux tile is held across 8 slots)
            pj_state = {}

            def qsingle(qb, p, c):
                def f():
                    if c == 0:
                        pj_state[(qb, p)] = aux_pool.tile(
                            [128, 512], F32, tag="aux",
                            name=f"qpj{qb}_{p}")
                    ps = pj_state[(qb, p)]
                    nc.tensor.matmul(
                        ps[:],
                        lhsT=wq_sb[:, c * 512 + p * 128:
                                   c * 512 + (p + 1) * 128],
                        rhs=qstg[qb][c][:],
                        start=(c == 0), stop=(c == NC_CHUNKS - 1))
                    if c == NC_CHUNKS - 1:
                        nc.vector.tensor_copy(qhTb[p][qb][:], ps[:])
                return f

            def qb0_fillers():
                for kb in range(1, NQB):
                    kstg[kb] = stage_block(kT, kb)
                # None: no filler in the very first g iteration - lets
                # the attention pipeline warm up before the sc ring is
                # borrowed for projection accumulators.
                fills = [None, kf(1, 0), kf(0, 1), kf(2, 0), kf(0, 2),
                         kf(3, 0), kf(0, 3), qf(0, 1, sc_pool), kf(1, 1),
                         kf(2, 1), kf(3, 1), qf(0, 2, sc_pool), kf(1, 2),
                         kf(2, 2), kf(3, 2), qf(0, 3, sc_pool), kf(1, 3),
                         kf(2, 3), kf(3, 3)]

                def stage_q1():
                    qstg[1] = stage_block(qT, 1)
                fills.append(stage_q1)
                for p in range(NP):
                    fills.append(qf(1, p, sc_pool))
                return fills

            # completed blocks' Wo matmuls wait in a FIFO consumed across
            # later blocks' filler slots: qb1/qb2 are PE-bound (fillers
            # cost wall-clock 1:1) while qb3 has ACT-bound slack where
            # deferred Wo work is nearly free. Caps are multiples of 4 so
            # a Wo accumulation group never splits across blocks.
            wo_queue = []
            WO_CAP = {1: 20, 2: 24, 3: 10 ** 6}

            def qbn_fillers(qb):
                fills = []
                has_q = qb + 1 < NQB
                if has_q:
                    def stage_qn():
                        qstg[qb + 1] = stage_block(qT, qb + 1)
                    fills.append(stage_qn)
                nwo = min(WO_CAP[qb], len(wo_queue))
                wos = [wo_queue.pop(0) for _ in range(nwo)]
                qi = 0
                # interleave: one 4-matmul Wo group, then one 8-matmul
                # q-projection group (the two aux slots alternate)
                while wos or (has_q and qi < NP):
                    fills += wos[:4]
                    del wos[:4]
                    if has_q and qi < NP:
                        for c in range(NC_CHUNKS):
                            fills.append(qsingle(qb + 1, qi, c))
                        qi += 1
                return fills

            prev_normT = None
            for qb in range(NQB):
                fills = (qb0_fillers() if qb == 0
                         else qbn_fillers(qb))
                fills = iter(fills)

                def filler():
                    f = next(fills, None)
                    if f is not None and callable(f):
                        f()

                normT = []
                for p in range(NP):
                    h0, h1 = 2 * p, 2 * p + 1
                    mix2 = mix_pool.tile([128, 1024], F32, tag="mix")
                    mixP = mix2[:, 0:512]
                    mixR = mix2[:, 512:1024]
                    l0 = slice(h0 * 65, h0 * 65 + 65)
                    l1 = slice(h1 * 65, h1 * 65 + 65)

                    def mix_g(g, exs):
                        ex0, ex1 = exs
                        for s2 in range(2):
                            kc = 2 * g + s2
                            esl = slice(s2 * 512, (s2 + 1) * 512)
                            nc.tensor.matmul(
                                mixP[0:65, :],
                                lhsT=vhs[kc][:, l0], rhs=ex0[:, esl],
                                start=(kc == 0), stop=(kc == NKC - 1))
                        for s2 in range(2):
                            kc = 2 * g + s2
                            esl = slice(s2 * 512, (s2 + 1) * 512)
                            nc.tensor.matmul(
                                mixR[0:65, :],
                                lhsT=vhs[kc][:, l1], rhs=ex1[:, esl],
                                start=(kc == 0), stop=(kc == NKC - 1))

                    # mix for group g is issued one iteration behind its
                    # exp, so the PE never sits adjacent to the ACT
                    # dependency (weights prefetch, no stall).
                    pend = None
                    for g in range(NKC // 2):
                        filler()
                        if not (qb == 0 and p == 0):
                            filler()
                        sc0 = sc_pool.tile([128, 1024], F32, tag="sc")
                        sc1 = sc_pool.tile([128, 1024], F32, tag="sc")
                        for s2 in range(2):
                            kc = 2 * g + s2
                            kb, ko = kc // 4, kc % 4
                            ksl = slice(ko * 128, (ko + 1) * 128)
                            nc.tensor.matmul(
                                sc0[:, s2 * 512:(s2 + 1) * 512],
                                lhsT=khTb[p][kb][0:64, ksl],
                                rhs=qhTb[p][qb][0:64, :],
                                start=True, stop=True)
                            nc.tensor.matmul(
                                sc1[:, s2 * 512:(s2 + 1) * 512],
                                lhsT=khTb[p][kb][64:128, ksl],
                                rhs=qhTb[p][qb][64:128, :],
                                start=True, stop=True)
                        if qb == 0 and p == 0:
                            # project v chunks just-in-time for mix;
                            # prefetch the next vT quarter (ring of 2)
                            if g == 2:
                                stage_vq(2)
                            elif g == 4:
                                stage_vq(3)
                            vproj_chunk(2 * g)
                            vproj_chunk(2 * g + 1)
                        ex0 = exp_pool.tile([128, 1024], BF16, tag="exp")
                        ex1 = exp_pool.tile([128, 1024], BF16, tag="exp")
                        nc.scalar.activation(ex0[:], sc0[:], EXP)
                        nc.scalar.activation(ex1[:], sc1[:], EXP)
                        if pend is not None:
                            mix_g(g - 1, pend)
                        pend = (ex0, ex1)
                    mix_g(NKC // 2 - 1, pend)
                    nt = norm_pool.tile([128, 512], BF16, tag="norm")
                    normT.append(nt)
                    if qb == NQB - 1 and p == NP - 1:
                        # kernel tail: lowest-latency normalize. Broadcast
                        # the sums row with a K=1 f32r matmul (PE is idle
                        # here), multiply straight out of PSUM on DVE.
                        su_r = sums_pool.tile([65, 1024], F32R, tag="sumr",
                                              name="sur")
                        nc.vector.tensor_copy(su_r[64:65, :], mix2[64:65, :])
                        bcp = sc_pool.tile([128, 1024], F32, tag="sc",
                                           name="bcp")
                        nc.tensor.matmul(
                            bcp[0:64, 0:512], lhsT=e_ones[64:65, :],
                            rhs=su_r[64:65, 0:512], start=True, stop=True)
                        nc.tensor.matmul(
                            bcp[0:64, 512:1024], lhsT=e_ones[64:65, :],
                            rhs=su_r[64:65, 512:1024], start=True, stop=True)
                        recb = rec_pool.tile([64, 1024], F32, tag="rec")
                        nc.vector.reciprocal_approx_fast(recb[:],
                                                         bcp[0:64, :])
                        nc.vector.tensor_mul(nt[0:64, :], mix2[0:64, 0:512],
                                             recb[:, 0:512])
                        sh1 = sh_pool.tile([64, 512], BF16, tag="sh1")
                        nc.vector.tensor_mul(sh1[:], mix2[0:64, 512:1024],
                                             recb[:, 512:1024])
                        nc.sync.dma_start(nt[64:128, :], sh1[:])
                    else:
                        # normalize (no PE involvement): evacuate mix rows
                        # 0:65 to SBUF (frees the PSUM tile), bounce the
                        # sums row through DRAM to broadcast it over
                        # partitions, reciprocal on DVE, scale on GpSimd.
                        madd = sums_pool.tile([65, 1024], F32, tag="sums")
                        nc.vector.tensor_copy(madd[:], mix2[0:65, :])
                        dsc = dram_pool.tile([1, 1024], F32, tag="dsc")
                        nc.sync.dma_start(dsc[:], madd[64:65, :])
                        rin = rec_pool.tile([64, 1024], F32, tag="rec")
                        nc.sync.dma_start(
                            rin[:], dsc[0:1, :].to_broadcast((64, 1024)))
                        recb = rec_pool.tile([64, 1024], F32, tag="rec")
                        nc.vector.reciprocal_approx_fast(recb[:], rin[:])
                        nc.gpsimd.tensor_mul(nt[0:64, :], madd[0:64, 0:512],
                                             recb[:, 0:512])
                        sh1 = sh_pool.tile([64, 512], BF16, tag="sh1")
                        nc.gpsimd.tensor_mul(sh1[:], madd[0:64, 512:1024],
                                             recb[:, 512:1024])
                        nc.sync.dma_start(nt[64:128, :], sh1[:])

                # drain any unissued fillers for this qb
                for f in fills:
                    if callable(f):
                        f()
                prev_normT = normT
                if qb < NQB - 1:
                    for tt in range(4):
                        for dh in range(2):
                            for p in range(NP):
                                wo_queue.append(
                                    lambda qb=qb, normT=normT, tt=tt,
                                    dh=dh, p=p: wo_single(
                                        qb, normT, tt, dh, p))

            # final block's Wo: 6 groups held concurrently across the
            # aux + sc PSUM banks, issued p-major so the p0-p2 matmuls
            # execute during the last pair's (ACT-bound) attention; only
            # the p3 matmuls + evacs trail the last normalize.
            combos = [(tt, dh) for tt in range(4) for dh in range(2)]
            wo_tiles = [
                aux_pool.tile([128, 512], F32, tag="aux", name="fwa0"),
                aux_pool.tile([128, 512], F32, tag="aux", name="fwa1"),
            ]
            fwsc0 = sc_pool.tile([128, 1024], F32, tag="sc", name="fwsc0")
            fwsc1 = sc_pool.tile([128, 1024], F32, tag="sc", name="fwsc1")
            wo_tiles += [fwsc0[:, 0:512], fwsc0[:, 512:1024],
                         fwsc1[:, 0:512], fwsc1[:, 512:1024]]
            for p in range(NP):
                for i in range(6):
                    tt, dh = combos[i]
                    nc.tensor.matmul(
                        wo_tiles[i][:],
                        lhsT=prev_normT[p][:, tt * 128:(tt + 1) * 128],
                        rhs=wo_sb[:, p * 1024 + dh * 512:
                                  p * 1024 + (dh + 1) * 512],
                        start=(p == 0), stop=(p == NP - 1))
            for i in range(6):
                tt, dh = combos[i]
                osb = osb_pool.tile([128, 512], F32, tag="osb")
                nc.vector.tensor_copy(osb[:], wo_tiles[i][:])
                nc.sync.dma_start(
                    out[(NQB - 1) * 512 + tt * 128:
                        (NQB - 1) * 512 + (tt + 1) * 128,
                        dh * 512:(dh + 1) * 512], osb[:])
            for tt, dh in combos[6:]:
                wo_piece(NQB - 1, prev_normT, tt, dh)

    nc.compile()
    return nc


def _get_nc():
    if "nc" not in _COMPILED:
        _COMPILED["nc"] = _build_nc()
    return _COMPILED["nc"]


def _shard_inputs(q, k, v, mask, Wq, Wk, Wv, Wo):
    """Build the per-core input maps (host-side layout prep)."""
    import ml_dtypes

    bf16 = ml_dtypes.bfloat16
    in_maps = []
    maskf = np.asarray(mask).astype(np.float32)
    q = np.asarray(q, np.float32)
    k = np.asarray(k, np.float32)
    v = np.asarray(v, np.float32)
    Wq = np.asarray(Wq, np.float32)
    Wk = np.asarray(Wk, np.float32)
    Wv = np.asarray(Wv, np.float32)
    Wo = np.asarray(Wo, np.float32)
    scale = np.float32(1.0 / np.sqrt(DK))
    for c in range(NCORES):
        b, hg = c // 2, c % 2
        hs = hg * HC
        m = {
            "qT": np.ascontiguousarray(q[b].T).astype(bf16),
            "kT": np.ascontiguousarray(k[b].T).astype(bf16),
            "vT": np.ascontiguousarray(v[b].T).astype(bf16),
            # head-major col blocks; fold 1/sqrt(dk) into Wq
            "wq": np.ascontiguousarray(
                Wq[hs:hs + HC].transpose(1, 0, 2).reshape(D, HC * DK) * scale
            ).astype(bf16),
            "wk": np.ascontiguousarray(
                Wk[hs:hs + HC].transpose(1, 0, 2).reshape(D, HC * DK)
            ).astype(bf16),
            "wv": np.ascontiguousarray(
                Wv[hs:hs + HC].transpose(1, 0, 2).reshape(D, HC * DV)
            ).astype(bf16),
            "wo": np.ascontiguousarray(Wo[hs * DV:(hs + HC) * DV]).astype(bf16),
            "maskr": np.ascontiguousarray(
                maskf[b].reshape(NKC, 128).T).astype(np.float32),
        }
        in_maps.append(m)
    return in_maps


def kernel(q, k, v, mask, Wq, Wk, Wv, Wo, _trace=False):
    from concourse.bass_utils import run_bass_kernel_spmd

    nc = _get_nc()
    in_maps = _shard_inputs(q, k, v, mask, Wq, Wk, Wv, Wo)
    res = run_bass_kernel_spmd(nc, in_maps, list(range(NCORES)),
                               trace=_trace)
    out = np.zeros((B, S, D), np.float32)
    for c in range(NCORES):
        out[c // 2] += res.results[c]["out"]
    if _trace:
        _COMPILED["last_result"] = res
    return out



# revision 31
# speedup vs baseline: 1.0129x; 1.0129x over previous
"""Multi-head attention (B=4, S=2048, D=1024, H=16, dk=dv=64) on 8 TRN2 cores.

Sharding: core c = 2*b + hg handles batch b = c//2 and heads
[hg*8, hg*8+8). Each core computes a partial output (its 8 heads'
contribution through Wo); the host adds the two partials per batch.

v2 design (vs the serial-matmul baseline): the PE array is addressed in
32-strip tiles so the half-array attention matmuls run CONCURRENTLY
(hw-measured 2.0x):

  - scores: K=dk=64 -> the two heads of a pair run as a row-tiled wave
    (h0 in array rows 0-63, h1 in rows 64-127), both N=512 matmuls
    retire in ~216ns total. Outputs land in one [128,1024] PSUM tile
    (h0 cols 0:512 | h1 cols 512:1024) so ONE ScalarE exp instruction
    covers the pair.
  - mix: M=dv=64 -> col-tiled wave: h0's mix into PSUM partitions 0:64
    (array cols 0-63), h1's into 64:128, accumulating over the 16 key
    chunks into a single 1-bank [128,512] tile that is ALREADY in the
    normalized-lhsT layout Wo wants.
  - softmax denominators: one 4-way col-tiled PE wave per key chunk
    (M=1 mask column per head at array cols 0/32/64/96, hw-measured
    ~217ns for all 4 heads) accumulating [1,512] rows into a single
    PSUM bank at partitions 0/32/64/96; DVE evacuates + reciprocal,
    GpSimd broadcasts across partitions (~0.9us), DVE multiplies the
    mix PSUM -> normalized bf16.

ScalarE runs only the 256 exp instructions (~280us) and is the
bottleneck engine; everything else is scheduled so exp never starves.
The loop is block-major (block = 2 pairs = 4 heads) so the k/v
projection build cost spreads over 4 query blocks of ACT time per
block: for blk in (pairs 01, pairs 23): for qb in 4: 16 key-chunk
iterations of [fillers | scores wave P | scores wave Q | exp P | exp Q
| mix waves (kc-1) | sums wave (kc-1)]; then normalize. Wo for query
block qb runs as fillers once blk1's normalize for qb lands
(accumulate over the 4 pairs, K=128).

Fillers (k/q/v projections, Wo) are deadline-ordered thunks consumed
at the top of each kc iteration; they accumulate in half-bank
[128,256] PSUM tiles (ping-pong in one bank) with evacs on DVE.

PSUM: sc ring 2x[128,1024] (4 banks), mix 2x[128,512] (2 banks),
sums 1x[128,512] (1 bank), aux 2x[128,256] (1 bank).
"""

import numpy as np

B, S, D = 4, 2048, 1024
H, DK, DV = 16, 64, 64
HC = 8          # heads per core
NP = HC // 2    # head pairs per core
NCORES = 8
NC_CHUNKS = D // 128    # 8 contraction chunks over D
NKC = S // 128          # 16 key chunks
NQB = S // 512          # 4 query blocks

_COMPILED = {}


def _build_nc():
    import concourse.tile as tile
    from concourse import bacc, mybir
    from contextlib import ExitStack

    F32 = mybir.dt.float32
    BF16 = mybir.dt.bfloat16
    EXP = mybir.ActivationFunctionType.Exp

    nc = bacc.Bacc("TRN2", target_bir_lowering=False, debug=False,
                   num_devices=NCORES)

    qT = nc.dram_tensor("qT", [D, S], BF16, kind="ExternalInput").ap()
    kT = nc.dram_tensor("kT", [D, S], BF16, kind="ExternalInput").ap()
    vT = nc.dram_tensor("vT", [D, S], BF16, kind="ExternalInput").ap()
    wq = nc.dram_tensor("wq", [D, HC * DK], BF16, kind="ExternalInput").ap()
    wk = nc.dram_tensor("wk", [D, HC * DK], BF16, kind="ExternalInput").ap()
    wv = nc.dram_tensor("wv", [D, HC * DV], BF16, kind="ExternalInput").ap()
    wo = nc.dram_tensor("wo", [HC * DV, D], BF16, kind="ExternalInput").ap()
    maskr = nc.dram_tensor("maskr", [128, NKC], F32, kind="ExternalInput").ap()
    out = nc.dram_tensor("out", [S, D], F32, kind="ExternalOutput").ap()

    with tile.TileContext(nc) as tc:
        with ExitStack() as ctx:
            const_pool = ctx.enter_context(tc.tile_pool(name="const", bufs=1))
            w_pool = ctx.enter_context(tc.tile_pool(name="weights", bufs=1))
            act_pool = ctx.enter_context(tc.tile_pool(name="acts", bufs=1))
            kst_pool = ctx.enter_context(
                tc.tile_pool(name="kstage", bufs=4 * NC_CHUNKS))
            qst_pool = ctx.enter_context(
                tc.tile_pool(name="qstage", bufs=2 * NC_CHUNKS))
            vt_pool = ctx.enter_context(tc.tile_pool(name="vstage", bufs=2))
            exp_pool = ctx.enter_context(tc.tile_pool(name="exp", bufs=6))
            nt_pool = ctx.enter_context(tc.tile_pool(name="norm", bufs=12))
            sums_sb_pool = ctx.enter_context(
                tc.tile_pool(name="sumssb", bufs=4))
            recb_pool = ctx.enter_context(tc.tile_pool(name="recb", bufs=4))
            osb_pool = ctx.enter_context(tc.tile_pool(name="outsb", bufs=6))
            sc_pool = ctx.enter_context(
                tc.tile_pool(name="scpsum", bufs=2, space="PSUM"))
            mix_pool = ctx.enter_context(
                tc.tile_pool(name="mxpsum", bufs=2, space="PSUM"))
            sums_pool = ctx.enter_context(
                tc.tile_pool(name="smpsum", bufs=1, space="PSUM"))
            aux_pool = ctx.enter_context(
                tc.tile_pool(name="auxpsum", bufs=1, space="PSUM"))
            dram_pool = ctx.enter_context(
                tc.tile_pool(name="dscratch", bufs=4, space="DRAM"))

            mask_sb = const_pool.tile([128, NKC], F32)
            nc.sync.dma_start(mask_sb[:], maskr[:])
            mask_bf = const_pool.tile([128, NKC], BF16)
            nc.vector.tensor_copy(mask_bf[:], mask_sb[:])
            # PE warmup: dummy matmuls on resident SBUF data bridge the
            # initial weight/staging DMA wait at full HAM ramp.
            warm_sb = const_pool.tile([128, 512], BF16)
            nc.vector.memset(warm_sb[:], 0.0)
            warm_ps = sums_pool.tile([128, 512], F32, tag="sums",
                                     name="warmps")
            for i in range(26):
                nc.tensor.matmul(warm_ps[:], lhsT=warm_sb[:, 0:128],
                                 rhs=warm_sb[:],
                                 start=(i == 0), stop=(i == 25))
            warm_out = const_pool.tile([128, 512], F32)
            nc.vector.tensor_copy(warm_out[:], warm_ps[:])

            # DMA priority: wk + k staging first (head critical path),
            # then wq + q staging; wv before attention, wo much later.
            wq_sb = w_pool.tile([128, NC_CHUNKS * 512], BF16, tag="wq")
            wk_sb = w_pool.tile([128, NC_CHUNKS * 512], BF16, tag="wk")
            wv_sb = w_pool.tile([128, NC_CHUNKS * 512], BF16, tag="wv")
            wo_sb = w_pool.tile([128, NP * 1024], BF16, tag="wo")
            for c in range(NC_CHUNKS):
                nc.sync.dma_start(wk_sb[:, c * 512:(c + 1) * 512],
                                  wk[c * 128:(c + 1) * 128, :])

            # persistent activations
            qhTb = [[act_pool.tile([128, 512], BF16, tag=f"qhT{p}_{b}",
                                   name=f"qhT{p}_{b}") for b in range(NQB)]
                    for p in range(NP)]
            khT = [act_pool.tile([128, S], BF16, tag=f"khT{p}",
                                 name=f"khT{p}") for p in range(NP)]
            khTb = [[khT[p][:, b * 512:(b + 1) * 512] for b in range(NQB)]
                    for p in range(NP)]
            vhs_all = act_pool.tile([128, NKC * 512], BF16, tag="vhall")
            vhs = [vhs_all[:, t * 512:(t + 1) * 512] for t in range(NKC)]

            def stage_block(pool, src, blk):
                stg = []
                for c in range(NC_CHUNKS):
                    t = pool.tile([128, 512], BF16, tag="stage",
                                  name=f"stg{c}")
                    nc.sync.dma_start(
                        t[:],
                        src[c * 128:(c + 1) * 128,
                            blk * 512:(blk + 1) * 512])
                    stg.append(t)
                return stg

            # q/k projection: whole group = 8 K-chunks x the two
            # 256-token halves (sharing each lhsT load) into a pair of
            # half-bank aux accumulators; evacs at the end. Filler
            # groups run to completion (never interleaved) so the
            # 2-buffer aux pool can never head-of-line-block the PE.
            def proj_group_run(kind, stg, wsb, dst_tile, p, tag):
                ps = aux_pool.tile([128, 512], F32, tag="aux",
                                   name=f"pj{tag}")
                for c in range(NC_CHUNKS):
                    nc.tensor.matmul(
                        ps[:],
                        lhsT=wsb[:, c * 512 + p * 128:
                                 c * 512 + (p + 1) * 128],
                        rhs=stg[c][:],
                        start=(c == 0), stop=(c == NC_CHUNKS - 1))
                nc.vector.tensor_copy(dst_tile[:], ps[:])

            # v projection: per key chunk t and head-half (4 heads =
            # 256 cols), 8 matmuls N=256 into one aux accumulator.
            vt_cur = {}

            def vproj_group_run(t, half):
                vt = vt_cur[t // 4]
                o = t % 4
                aux = aux_pool.tile([128, 512], F32, tag="aux",
                                    name=f"vpj{t}_{half}")
                ps = aux[:, 0:256]
                for c in range(NC_CHUNKS):
                    nc.tensor.matmul(
                        ps,
                        lhsT=vt[:, c * 512 + o * 128:
                                c * 512 + (o + 1) * 128],
                        rhs=wv_sb[:, c * 512 + half * 256:
                                  c * 512 + (half + 1) * 256],
                        start=(c == 0), stop=(c == NC_CHUNKS - 1))
                # mask the value rows (masked keys contribute 0)
                nc.vector.tensor_scalar_mul(
                    vhs[t][:, half * 256:(half + 1) * 256],
                    ps, mask_sb[:, t:t + 1])

            def stage_vq(vq):
                vt = vt_pool.tile([128, NC_CHUNKS * 512], BF16, tag="vt",
                                  name=f"vq{vq}")
                for c in range(NC_CHUNKS):
                    nc.sync.dma_start(
                        vt[:, c * 512:(c + 1) * 512],
                        vT[c * 128:(c + 1) * 128,
                           vq * 512:(vq + 1) * 512])
                vt_cur[vq] = vt

            # Wo: one group = (tt, dh): accumulate over the 4 pairs
            # into a pair of half-bank tiles (halves share each lhsT
            # load), evac + DMA out at the end.
            def wo_group_run(qb, nts, tt, dh):
                wps = aux_pool.tile([128, 512], F32, tag="aux",
                                    name=f"wo{tt}{dh}")
                for p in range(NP):
                    nc.tensor.matmul(
                        wps[:],
                        lhsT=nts[p][:, tt * 128:(tt + 1) * 128],
                        rhs=wo_sb[:, p * 1024 + dh * 512:
                                  p * 1024 + (dh + 1) * 512],
                        start=(p == 0), stop=(p == NP - 1))
                osb = osb_pool.tile([128, 512], F32, tag="osb")
                nc.vector.tensor_copy(osb[:], wps[:])
                nc.sync.dma_start(
                    out[qb * 512 + tt * 128:qb * 512 + (tt + 1) * 128,
                        dh * 512:(dh + 1) * 512], osb[:])

            # ---- attention waves ----
            def scores_wave(p, qb, kc, sc):
                kb, ko = kc // 4, kc % 4
                ksl = slice(ko * 128, (ko + 1) * 128)
                nc.tensor.matmul(sc[:, 0:512],
                                 lhsT=khTb[p][kb][0:64, ksl],
                                 rhs=qhTb[p][qb][0:64, :],
                                 start=True, stop=True)
                nc.tensor.matmul(sc[:, 512:1024],
                                 lhsT=khTb[p][kb][64:128, ksl],
                                 rhs=qhTb[p][qb][64:128, :],
                                 start=True, stop=True)

            # mix/sums accumulate onto DVE-memset zeros with
            # start=False on every matmul: a col-tiled bank hosts two
            # accumulation streams, and a start_tensor_calc by either
            # would lazily re-zero the whole 2KB region under the
            # other's partial sums. Accumulating onto true zeros is
            # correct under both has_written semantics.
            def mix_wave(p, kc, ex, mixps):
                h0 = vhs[kc][:, (2 * p % HC) * 64:(2 * p % HC) * 64 + 64]
                h1 = vhs[kc][:, ((2 * p + 1) % HC) * 64:
                             ((2 * p + 1) % HC) * 64 + 64]
                nc.tensor.matmul(mixps[0:64, :], lhsT=h0, rhs=ex[:, 0:512],
                                 start=False, stop=False,
                                 skip_group_check=True,
                                 tile_position=(0, 0))
                nc.tensor.matmul(mixps[64:128, :], lhsT=h1,
                                 rhs=ex[:, 512:1024],
                                 start=False, stop=False,
                                 skip_group_check=True,
                                 tile_position=(0, 64))

            def sums_wave(kc, exA, exB, sums):
                mcol = mask_bf[:, kc:kc + 1]
                for j, rhs in enumerate((exA[:, 0:512], exA[:, 512:1024],
                                         exB[:, 0:512], exB[:, 512:1024])):
                    nc.tensor.matmul(sums[32 * j:32 * j + 1, :],
                                     lhsT=mcol, rhs=rhs,
                                     start=False, stop=False,
                                     skip_group_check=True,
                                     tile_position=(0, 32 * j))

            def recip_sums(sums, tag):
                # one reciprocal over the sums bank (rows 1-31 etc are
                # memset zeros -> inf, never read), PSUM -> SBUF.
                rec = sums_sb_pool.tile([97, 512], F32, tag="ssb",
                                        name=f"rec{tag}")
                nc.vector.reciprocal_approx_fast(rec[:], sums[0:97, :])
                return rec

            def normalize(mixps, rec, j0, pair_tag):
                # broadcast each head's reciprocal row over its 64
                # partitions via a DRAM bounce (partition_broadcast is
                # wrong on hw), multiply the mix PSUM -> bf16.
                dsc = dram_pool.tile([2, 512], F32, tag="dsc")
                nc.sync.dma_start(dsc[0:1, :], rec[32 * j0:32 * j0 + 1, :])
                nc.sync.dma_start(dsc[1:2, :],
                                  rec[32 * j0 + 32:32 * j0 + 33, :])
                recb = recb_pool.tile([128, 512], F32, tag="recb")
                nc.sync.dma_start(recb[0:64, :],
                                  dsc[0:1, :].to_broadcast((64, 512)))
                nc.sync.dma_start(recb[64:128, :],
                                  dsc[1:2, :].to_broadcast((64, 512)))
                nt = nt_pool.tile([128, 512], BF16, tag="norm",
                                  name=f"nt{pair_tag}")
                nc.vector.tensor_mul(nt[:], mixps[:], recb[:])
                return nt

            # ---- startup: stage + project what the first exps need ----
            kstg = [None] * NQB
            qstg = {}
            # DMA order is the head critical path: wk+k0, wq+q0 feed the
            # first four projection groups; wv/v-quarters are only
            # needed by the v-projections a few kc in.
            kstg[0] = stage_block(kst_pool, kT, 0)
            for c in range(NC_CHUNKS):
                nc.sync.dma_start(wq_sb[:, c * 512:(c + 1) * 512],
                                  wq[c * 128:(c + 1) * 128, :])
            qstg[0] = stage_block(qst_pool, qT, 0)
            proj_group_run("k", kstg[0], wk_sb, khTb[0][0], 0, "k00")
            proj_group_run("q", qstg[0], wq_sb, qhTb[0][0], 0, "q00")
            proj_group_run("k", kstg[0], wk_sb, khTb[1][0], 1, "k10")
            proj_group_run("q", qstg[0], wq_sb, qhTb[1][0], 1, "q10")
            for c in range(NC_CHUNKS):
                nc.sync.dma_start(wv_sb[:, c * 512:(c + 1) * 512],
                                  wv[c * 128:(c + 1) * 128, :])
            stage_vq(0)
            stage_vq(1)

            # ---- filler machinery: deadline-ordered group thunks ----
            def run_fills(fills, kc):
                for f in fills.pop(kc, []):
                    f()

            def add_fill(fills, kc, f):
                fills.setdefault(kc, []).append(f)

            def kproj_at(fills, kc0, p, kb):
                add_fill(fills, kc0,
                         lambda p=p, kb=kb: proj_group_run(
                             "k", kstg[kb], wk_sb, khTb[p][kb], p,
                             f"k{p}{kb}"))

            def qproj_at(fills, kc0, p, qb):
                add_fill(fills, kc0,
                         lambda p=p, qb=qb: proj_group_run(
                             "q", qstg[qb], wq_sb, qhTb[p][qb], p,
                             f"q{p}{qb}"))

            wo_queue = []

            def build_fills(blk, qb):
                fills = {}
                if qb == 0:
                    # k blocks 1-3 JIT (kstg[kb] staged ~4 kc ahead of
                    # the kproj at 4*kb-3/-2, needed by scores at 4*kb),
                    # v chunk halves JIT (vhs[kc] by the mix at iter
                    # kc+1, so deadline kc+1)
                    if blk == 0:
                        for kb in range(1, NQB):
                            def stage_kb(kb=kb):
                                kstg[kb] = stage_block(kst_pool, kT, kb)
                            add_fill(fills, max(0, 4 * kb - 8), stage_kb)

                        def stage_wo():
                            for p in range(NP):
                                nc.sync.dma_start(
                                    wo_sb[:, p * 1024:(p + 1) * 1024],
                                    wo[p * 128:(p + 1) * 128, :])
                        add_fill(fills, 12, stage_wo)
                    pA, pB = 2 * blk, 2 * blk + 1
                    for t in range(NKC):
                        add_fill(fills, t + 2,
                                 lambda t=t, blk=blk: vproj_group_run(t, blk))
                        if t == 6:
                            add_fill(fills, 4, lambda: stage_vq(2))
                        if t == 10:
                            add_fill(fills, 8, lambda: stage_vq(3))
                    for kb in range(1, NQB):
                        kproj_at(fills, 4 * kb - 2, pA, kb)
                        kproj_at(fills, 4 * kb - 1, pB, kb)
                else:
                    # steady state: consume queued Wo work
                    nwo = min(8, len(wo_queue))
                    for i in range(nwo):
                        add_fill(fills, 1 + (i * 14) // max(nwo, 1),
                                 wo_queue.pop(0))
                if blk == 0 and qb == NQB - 1:
                    # restage v quarters 0/1 for blk1-qb0's vproj
                    def restage_v01():
                        stage_vq(0)
                        stage_vq(1)
                    add_fill(fills, 4, restage_v01)
                # stage/project next qb's q for this block's pairs
                pA, pB = 2 * blk, 2 * blk + 1
                if qb + 1 < NQB:
                    nqb = qb + 1

                    def stage_qn(nqb=nqb):
                        qstg[nqb] = stage_block(qst_pool, qT, nqb)
                    add_fill(fills, 6, stage_qn)
                    qproj_at(fills, 8, pA, nqb)
                    qproj_at(fills, 12, pB, nqb)
                elif blk == 0:
                    # blk1-qb0 prerequisites: q block 0 for pairs 2,3
                    # AND k block 0 for pairs 2,3 (blk1 fills only
                    # cover k blocks 1-3)
                    def stage_q0():
                        qstg[0] = stage_block(qst_pool, qT, 0)
                    add_fill(fills, 6, stage_q0)
                    qproj_at(fills, 8, 2, 0)
                    qproj_at(fills, 12, 3, 0)
                    kproj_at(fills, 10, 2, 0)
                    kproj_at(fills, 14, 3, 0)
                return fills

            # ---- main loop ----
            ntbl = [[None] * NP for _ in range(NQB)]
            for blk in range(2):
                pA, pB = 2 * blk, 2 * blk + 1
                for qb in range(NQB):
                    fills = build_fills(blk, qb)
                    mixA = mix_pool.tile([128, 512], F32, tag="mix",
                                         name=f"mixA{blk}{qb}")
                    mixB = mix_pool.tile([128, 512], F32, tag="mix",
                                         name=f"mixB{blk}{qb}")
                    sums = sums_pool.tile([128, 512], F32, tag="sums",
                                          name=f"sums{blk}{qb}")
                    nc.vector.memset(mixA[:], 0.0)
                    nc.vector.memset(mixB[:], 0.0)
                    nc.vector.memset(sums[:], 0.0)
                    pend = []
                    for kc in range(NKC):
                        run_fills(fills, kc)
                        scA = sc_pool.tile([128, 1024], F32, tag="sc")
                        scores_wave(pA, qb, kc, scA)
                        scB = sc_pool.tile([128, 1024], F32, tag="sc")
                        scores_wave(pB, qb, kc, scB)
                        exA = exp_pool.tile([128, 1024], BF16, tag="exp")
                        nc.scalar.activation(exA[:], scA[:], EXP)
                        exB = exp_pool.tile([128, 1024], BF16, tag="exp")
                        nc.scalar.activation(exB[:], scB[:], EXP)
                        # mix/sums lag 2 kc behind exp so neither a slow
                        # v-projection nor the ACT dependency can stall
                        # the scores pipeline
                        if len(pend) == 2:
                            eA, eB = pend.pop(0)
                            mix_wave(pA, kc - 2, eA, mixA)
                            mix_wave(pB, kc - 2, eB, mixB)
                            sums_wave(kc - 2, eA, eB, sums)
                        pend.append((exA, exB))
                    # drain leftover fillers, then the last waves
                    for k in sorted(fills.keys()):
                        run_fills(fills, k)
                    for i, (eA, eB) in enumerate(pend):
                        mix_wave(pA, NKC - 2 + i, eA, mixA)
                        mix_wave(pB, NKC - 2 + i, eB, mixB)
                        sums_wave(NKC - 2 + i, eA, eB, sums)
                    rec = recip_sums(sums, f"{blk}_{qb}")
                    ntbl[qb][pA] = normalize(mixA, rec, 0, f"{pA}_{qb}")
                    ntbl[qb][pB] = normalize(mixB, rec, 2, f"{pB}_{qb}")
                    if blk == 1:
                        if qb < NQB - 1:
                            for tt in range(4):
                                for dh in range(2):
                                    wo_queue.append(
                                        lambda qb=qb, tt=tt, dh=dh:
                                        wo_group_run(qb, ntbl[qb], tt, dh))
                        else:
                            # tail: the attention psum banks are idle
                            # now - run all 8 Wo groups concurrently,
                            # p-major, one accumulator bank each
                            slots = []
                            for i in range(2):
                                sct = sc_pool.tile([128, 1024], F32,
                                                   tag="sc", name=f"fw{i}")
                                slots += [sct[:, 0:512], sct[:, 512:1024]]
                            for i in range(2):
                                mt = mix_pool.tile([128, 512], F32,
                                                   tag="mix", name=f"fwm{i}")
                                slots.append(mt[:])
                            st = sums_pool.tile([128, 512], F32,
                                                tag="sums", name="fws")
                            slots.append(st[:])
                            at = aux_pool.tile([128, 512], F32,
                                               tag="aux", name="fwa")
                            slots.append(at[:])
                            combos = [(tt, dh) for tt in range(4)
                                      for dh in range(2)]
                            nts = ntbl[qb]
                            for p in range(NP):
                                for i, (tt, dh) in enumerate(combos):
                                    nc.tensor.matmul(
                                        slots[i],
                                        lhsT=nts[p][:, tt * 128:
                                                    (tt + 1) * 128],
                                        rhs=wo_sb[:, p * 1024 + dh * 512:
                                                  p * 1024 +
                                                  (dh + 1) * 512],
                                        start=(p == 0), stop=(p == NP - 1))
                            for i, (tt, dh) in enumerate(combos):
                                osb = osb_pool.tile([128, 512], F32,
                                                    tag="osb")
                                nc.vector.tensor_copy(osb[:], slots[i])
                                nc.sync.dma_start(
                                    out[qb * 512 + tt * 128:
                                        qb * 512 + (tt + 1) * 128,
                                        dh * 512:(dh + 1) * 512], osb[:])
            for f in wo_queue:
                f()

    nc.compile()
    return nc


def _get_nc():
    if "nc" not in _COMPILED:
        _COMPILED["nc"] = _build_nc()
    return _COMPILED["nc"]


def _shard_inputs(q, k, v, mask, Wq, Wk, Wv, Wo):
    """Build the per-core input maps (host-side layout prep)."""
    import ml_dtypes

    bf16 = ml_dtypes.bfloat16
    in_maps = []
    maskf = np.asarray(mask).astype(np.float32)
    q = np.asarray(q, np.float32)
    k = np.asarray(k, np.float32)
    v = np.asarray(v, np.float32)
    Wq = np.asarray(Wq, np.float32)
    Wk = np.asarray(Wk, np.float32)
    Wv = np.asarray(Wv, np.float32)
    Wo = np.asarray(Wo, np.float32)
    scale = np.float32(1.0 / np.sqrt(DK))
    for c in range(NCORES):
        b, hg = c // 2, c % 2
        hs = hg * HC
        m = {
            "qT": np.ascontiguousarray(q[b].T).astype(bf16),
            "kT": np.ascontiguousarray(k[b].T).astype(bf16),
            "vT": np.ascontiguousarray(v[b].T).astype(bf16),
            # head-major col blocks; fold 1/sqrt(dk) into Wq
            "wq": np.ascontiguousarray(
                Wq[hs:hs + HC].transpose(1, 0, 2).reshape(D, HC * DK) * scale
            ).astype(bf16),
            "wk": np.ascontiguousarray(
                Wk[hs:hs + HC].transpose(1, 0, 2).reshape(D, HC * DK)
            ).astype(bf16),
            "wv": np.ascontiguousarray(
                Wv[hs:hs + HC].transpose(1, 0, 2).reshape(D, HC * DV)
            ).astype(bf16),
            "wo": np.ascontiguousarray(Wo[hs * DV:(hs + HC) * DV]).astype(bf16),
            "maskr": np.ascontiguousarray(
                maskf[b].reshape(NKC, 128).T).astype(np.float32),
        }
        in_maps.append(m)
    return in_maps


def kernel(q, k, v, mask, Wq, Wk, Wv, Wo, _trace=False):
    from concourse.bass_utils import run_bass_kernel_spmd

    nc = _get_nc()
    in_maps = _shard_inputs(q, k, v, mask, Wq, Wk, Wv, Wo)
    res = run_bass_kernel_spmd(nc, in_maps, list(range(NCORES)),
                               trace=_trace)
    out = np.zeros((B, S, D), np.float32)
    for c in range(NCORES):
        out[c // 2] += res.results[c]["out"]
    if _trace:
        _COMPILED["last_result"] = res
    return out
